# revision 5
# baseline (speedup 1.0000x reference)
"""NVFP4 block-scaled matmul (A @ B^T + bias) on 8 TRN2 NeuronCores.

Strategy (tensor-parallel over N):
  - Host marshalling: decode b's packed fp4 codes to e4m3 value bytes
    (exact), pre-transposed to k-major [K, N/8] per core; b_scale
    transposed to [K/16, N/8] bf16; A side is tiny (64x smaller than B)
    so it is fully dequantized on host to bf16 [K, M] with the global
    scales folded in; bias replicated to [128, N/8] bf16.
  - Device kernel (per core): stream 64 k-chunks [128, NB]:
      DMA e4m3 bytes -> ACT fp8->bf16 convert -> DVE multiply by
      per-block scales (scales replicated 16x across partitions via a
      broadcast SBUF->SBUF DMA) -> PE matmul accumulating 8 [128,512]
      f32 PSUM tiles across all chunks -> bias add -> bf16 out.
"""

import numpy as np
import ml_dtypes

import concourse.bass as bass
import concourse.mybir as mybir
import concourse.tile as tile
from concourse import bacc
from concourse import bass_utils

P = 128
M, N, K = 256, 16384, 8192
NCORES = 8
NB = N // NCORES          # 2048  per-core N slab
KCH = K // P              # 64    k-chunks of 128
BLOCK = 16                # NVFP4 block size

_FP4 = np.array([0.0, 0.5, 1.0, 1.5, 2.0, 3.0, 4.0, 6.0,
                 -0.0, -0.5, -1.0, -1.5, -2.0, -3.0, -4.0, -6.0], np.float32)


def _codes(x_int32: np.ndarray) -> np.ndarray:
    """[rows, K//2] int32 byte values -> [rows, K] uint8 fp4 codes
    (low nibble first, matching the reference)."""
    b = x_int32.astype(np.uint8)
    lo = b & 0xF
    hi = b >> 4
    return np.stack([lo, hi], axis=-1).reshape(b.shape[0], -1)


def k_perm(kch: int) -> np.ndarray:
    """Row permutation applied on host: partition p of chunk c holds
    original k-row c*128 + (p % 8)*16 + p//8."""
    p = np.arange(P)
    within = (p % 8) * 16 + p // 8
    return (np.arange(kch)[:, None] * P + within[None, :]).reshape(-1)


def tile_body(tc, out_ap, at_ap, bt_ap, sbt_ap, bias_ap, *, kch=KCH, nb=NB, m=M):
    """Per-core kernel body. Shapes:
      at_ap  [kch*128, m]   bf16   A' transposed (dequant, k-major)
      bt_ap  [kch*128, nb]  uint8  e4m3 value bytes of B, k-major
      sbt_ap [kch*8,  nb]   bf16   b_scale transposed (kb-major)
      bias_ap [128, nb]     bf16   bias slab replicated across partitions
      out_ap [m, nb]        bf16
    """
    nc = tc.nc
    assert m % P == 0
    mh = m // P               # m subtiles (2)
    nq = nb // 512            # psum-width quarters (4)
    srows = kch * 8           # total scale rows
    sp = min(srows, P)        # scale slab partition dim
    so = srows // sp

    with (
        tc.tile_pool(name="const", bufs=1) as const,
        tc.tile_pool(name="bv", bufs=4) as bv_pool,
        tc.tile_pool(name="srep", bufs=4) as srep_pool,
        tc.tile_pool(name="bp", bufs=4) as bp_pool,
        tc.tile_pool(name="psum", bufs=1, space="PSUM") as psum_pool,
        tc.tile_pool(name="outp", bufs=2) as out_pool,
    ):
        # Resident tensors
        a_sb = const.tile([P, kch, m], mybir.dt.bfloat16, name="a_sb")
        nc.sync.dma_start(a_sb, at_ap.rearrange("(c p) m -> p c m", p=P))
        s_sb = const.tile([sp, so, nb], mybir.dt.bfloat16, name="s_sb")
        nc.sync.dma_start(s_sb, sbt_ap.rearrange("(o p) n -> p o n", p=sp))
        bias_sb = const.tile([P, nb], mybir.dt.bfloat16, name="bias_sb")
        nc.sync.dma_start(bias_sb, bias_ap)

        psums = [
            psum_pool.tile([P, 512], mybir.dt.float32, name=f"ps_{h}_{q}")
            for h in range(mh) for q in range(nq)
        ]

        bt3 = bt_ap.rearrange("(c p) n -> c p n", p=P)
        for c in range(kch):
            # raw e4m3 value bytes for this k-chunk
            bv = bv_pool.tile([P, nb], mybir.dt.uint8, name="bv")
            nc.sync.dma_start(bv, bt3[c])

            # replicate the 8 scale rows of this chunk 16x across partitions
            # (interleaved: srep[p] = scale row (p mod 8); the host permutes
            # k-rows within each chunk to match) via log-doubling copies
            p0 = (8 * c) % sp
            o0 = (8 * c) // sp
            srep = srep_pool.tile([P, nb], mybir.dt.bfloat16, name="srep")
            nc.sync.dma_start(srep[0:8], s_sb[p0:p0 + 8, o0, :])
            w = 8
            while w < P:
                nc.sync.dma_start(srep[w:2 * w], srep[0:w])
                w *= 2

            # fp8 -> bf16 on ACT, then scale on DVE
            bp = bp_pool.tile([P, nb], mybir.dt.bfloat16, name="bp")
            nc.scalar.copy(bp, bv.bitcast(mybir.dt.float8e4))
            nc.vector.tensor_mul(out=bp, in0=bp, in1=srep)

            first = c == 0
            last = c == kch - 1
            for h in range(mh):
                for q in range(nq):
                    nc.tensor.matmul(
                        psums[h * nq + q],
                        lhsT=a_sb[:, c, h * P:(h + 1) * P],
                        rhs=bp[:, q * 512:(q + 1) * 512],
                        start=first,
                        stop=last,
                    )

        # epilogue: bias add + cast to bf16 + store
        for h in range(mh):
            for q in range(nq):
                ot = out_pool.tile([P, 512], mybir.dt.bfloat16, name="ot")
                nc.vector.tensor_add(
                    out=ot,
                    in0=psums[h * nq + q],
                    in1=bias_sb[:, q * 512:(q + 1) * 512],
                )
                nc.sync.dma_start(
                    out_ap[h * P:(h + 1) * P, q * 512:(q + 1) * 512], ot
                )


def build(kch=KCH, nb=NB, m=M):
    nc = bacc.Bacc(
        "TRN2",
        target_bir_lowering=False,
        debug=False,
        num_devices=NCORES,
    )
    at = nc.dram_tensor("at", [kch * P, m], mybir.dt.bfloat16, kind="ExternalInput").ap()
    bt = nc.dram_tensor("bt", [kch * P, nb], mybir.dt.uint8, kind="ExternalInput").ap()
    sbt = nc.dram_tensor("sbt", [kch * 8, nb], mybir.dt.bfloat16, kind="ExternalInput").ap()
    bias = nc.dram_tensor("bias", [P, nb], mybir.dt.bfloat16, kind="ExternalInput").ap()
    out = nc.dram_tensor("out", [m, nb], mybir.dt.bfloat16, kind="ExternalOutput").ap()
    with tile.TileContext(nc) as tc:
        tile_body(tc, out, at, bt, sbt, bias, kch=kch, nb=nb, m=m)
    nc.compile()
    return nc


def marshal(a, a_scale, a_global_scale, b, b_scale, b_global_scale, bias):
    """Host-side input prep. Returns per-core in_maps."""
    a = np.asarray(a)
    a_scale = np.asarray(a_scale, np.float32)
    ga = float(np.asarray(a_global_scale, np.float32))
    b = np.asarray(b)
    b_scale = np.asarray(b_scale, np.float32)
    gb = float(np.asarray(b_global_scale, np.float32))
    bias = np.asarray(bias, np.float32)

    # A side: full dequant (small), fold global scales, transpose to [K, M]
    a_vals = _FP4[_codes(a)]                                   # [M, K]
    a_deq = a_vals.reshape(M, K // BLOCK, BLOCK) * (a_scale * (ga * gb))[..., None]
    at = np.ascontiguousarray(a_deq.reshape(M, K).T).astype(ml_dtypes.bfloat16)

    # B side: decode codes to e4m3 value bytes, transpose to [K, N]
    b_vals_e4m3 = _FP4.astype(ml_dtypes.float8_e4m3)[_codes(b)]   # [N, K] e4m3
    btf = np.ascontiguousarray(b_vals_e4m3.T).view(np.uint8)      # [K, N] u8

    # within-chunk k-row permutation: partition p holds original row
    # (p % 8) * 16 + p // 8, so its scale row is (p % 8)
    perm = k_perm(K // P)
    at = np.ascontiguousarray(at[perm])
    btf = np.ascontiguousarray(btf[perm])
    sbt_f = np.ascontiguousarray(b_scale.T).astype(ml_dtypes.bfloat16)  # [K/16, N]

    in_maps = []
    for ci in range(NCORES):
        sl = slice(ci * NB, (ci + 1) * NB)
        bias_rep = np.ascontiguousarray(
            np.broadcast_to(bias[None, sl], (P, NB))
        ).astype(ml_dtypes.bfloat16)
        in_maps.append({
            "at": at,
            "bt": np.ascontiguousarray(btf[:, sl]),
            "sbt": np.ascontiguousarray(sbt_f[:, sl]),
            "bias": bias_rep,
        })
    return in_maps


_CACHE = {}


def kernel(a, a_scale, a_global_scale, b, b_scale, b_global_scale, bias):
    in_maps = marshal(a, a_scale, a_global_scale, b, b_scale, b_global_scale, bias)
    if "nc" not in _CACHE:
        _CACHE["nc"] = build()
    res = bass_utils.run_bass_kernel_spmd(
        _CACHE["nc"], in_maps, core_ids=list(range(NCORES))
    )
    return np.concatenate([r["out"] for r in res.results], axis=1)


# revision 8
# speedup vs baseline: 98.1511x; 98.1511x over previous
"""NVFP4 block-scaled matmul (A @ B^T + bias) on 8 TRN2 NeuronCores.

Strategy (tensor-parallel over N):
  - Host marshalling: decode b's packed fp4 codes to e4m3 value bytes
    (exact), pre-transposed to k-major [K, N/8] per core; b_scale
    transposed to [K/16, N/8] bf16; A side is tiny (64x smaller than B)
    so it is fully dequantized on host to bf16 [K, M] with the global
    scales folded in; bias replicated to [128, N/8] bf16.
  - Device kernel (per core): stream 64 k-chunks [128, NB]:
      DMA e4m3 bytes -> ACT fp8->bf16 convert -> DVE multiply by
      per-block scales (scales replicated 16x across partitions via a
      broadcast SBUF->SBUF DMA) -> PE matmul accumulating 8 [128,512]
      f32 PSUM tiles across all chunks -> bias add -> bf16 out.
"""

import numpy as np
import ml_dtypes

import concourse.bass as bass
import concourse.mybir as mybir
import concourse.tile as tile
from concourse import bacc
from concourse import bass_utils

P = 128
M, N, K = 256, 16384, 8192
NCORES = 8
NB = N // NCORES          # 2048  per-core N slab
KCH = K // P              # 64    k-chunks of 128
BLOCK = 16                # NVFP4 block size

_FP4 = np.array([0.0, 0.5, 1.0, 1.5, 2.0, 3.0, 4.0, 6.0,
                 -0.0, -0.5, -1.0, -1.5, -2.0, -3.0, -4.0, -6.0], np.float32)


def _codes(x_int32: np.ndarray) -> np.ndarray:
    """[rows, K//2] int32 byte values -> [rows, K] uint8 fp4 codes
    (low nibble first, matching the reference)."""
    b = x_int32.astype(np.uint8)
    lo = b & 0xF
    hi = b >> 4
    return np.stack([lo, hi], axis=-1).reshape(b.shape[0], -1)


def k_perm(kch: int) -> np.ndarray:
    """Row permutation applied on host: partition p of chunk c holds
    original k-row c*128 + (p % 8)*16 + p//8."""
    p = np.arange(P)
    within = (p % 8) * 16 + p // 8
    return (np.arange(kch)[:, None] * P + within[None, :]).reshape(-1)


def tile_body(tc, out_ap, at_ap, bt_ap, sbt_ap, bias_ap, *, kch=KCH, nb=NB, m=M,
              repeat=1):
    """Per-core kernel body. Shapes:
      at_ap  [kch*128, m]   bf16   A' transposed (dequant, k-major)
      bt_ap  [kch*128, nb]  uint8  e4m3 value bytes of B, k-major
      sbt_ap [kch*8,  nb]   bf16   b_scale transposed (kb-major)
      bias_ap [128, nb]     bf16   bias slab replicated across partitions
      out_ap [m, nb]        bf16
    """
    nc = tc.nc
    assert m % P == 0
    mh = m // P               # m subtiles (2)
    nq = nb // 512            # psum-width quarters (4)
    srows = kch * 8           # total scale rows
    sp = min(srows, P)        # scale slab partition dim
    so = srows // sp

    with (
        tc.tile_pool(name="const", bufs=1) as const,
        tc.tile_pool(name="bv", bufs=4) as bv_pool,
        tc.tile_pool(name="srep", bufs=4) as srep_pool,
        tc.tile_pool(name="bp", bufs=4) as bp_pool,
        tc.tile_pool(name="psum", bufs=1, space="PSUM") as psum_pool,
        tc.tile_pool(name="outp", bufs=2) as out_pool,
    ):
        # Resident tensors
        a_sb = const.tile([P, kch, m], mybir.dt.bfloat16, name="a_sb")
        nc.sync.dma_start(a_sb, at_ap.rearrange("(c p) m -> p c m", p=P))
        s_sb = const.tile([sp, so, nb], mybir.dt.bfloat16, name="s_sb")
        nc.sync.dma_start(s_sb, sbt_ap.rearrange("(o p) n -> p o n", p=sp))
        bias_sb = const.tile([P, nb], mybir.dt.bfloat16, name="bias_sb")
        nc.sync.dma_start(bias_sb, bias_ap)

        def body():
            _pipeline(tc, out_ap, bt_ap, a_sb, s_sb, bias_sb,
                      kch=kch, nb=nb, m=m, sp=sp,
                      bv_pool=bv_pool, srep_pool=srep_pool, bp_pool=bp_pool,
                      psum_pool=psum_pool, out_pool=out_pool)

        if repeat == 1:
            body()
        else:
            with tc.For_i(0, repeat, 1,
                          hint_engines=(mybir.EngineType.PE,
                                        mybir.EngineType.Activation,
                                        mybir.EngineType.DVE,
                                        mybir.EngineType.Pool,
                                        mybir.EngineType.SP)):
                body()


def _pipeline(tc, out_ap, bt_ap, a_sb, s_sb, bias_sb, *, kch, nb, m, sp,
              bv_pool, srep_pool, bp_pool, psum_pool, out_pool):
        nc = tc.nc
        mh = m // P
        nq = nb // 512
        psums = [
            psum_pool.tile([P, 512], mybir.dt.float32, name=f"ps_{h}_{q}")
            for h in range(mh) for q in range(nq)
        ]

        bt3 = bt_ap.rearrange("(c p) n -> c p n", p=P)
        for c in range(kch):
            # raw e4m3 value bytes for this k-chunk
            bv = bv_pool.tile([P, nb], mybir.dt.uint8, name="bv")
            nc.sync.dma_start(bv, bt3[c])

            # replicate the 8 scale rows of this chunk 16x across partitions
            # (interleaved: srep[p] = scale row (p mod 8); the host permutes
            # k-rows within each chunk to match) via log-doubling copies
            p0 = (8 * c) % sp
            o0 = (8 * c) // sp
            srep = srep_pool.tile([P, nb], mybir.dt.bfloat16, name="srep")
            nc.sync.dma_start(srep[0:8], s_sb[p0:p0 + 8, o0, :])
            w = 8
            while w < P:
                nc.sync.dma_start(srep[w:2 * w], srep[0:w])
                w *= 2

            # fp8 -> bf16 on ACT, then scale on DVE
            bp = bp_pool.tile([P, nb], mybir.dt.bfloat16, name="bp")
            nc.scalar.copy(bp, bv.bitcast(mybir.dt.float8e4))
            nc.vector.tensor_mul(out=bp, in0=bp, in1=srep)

            first = c == 0
            last = c == kch - 1
            for h in range(mh):
                for q in range(nq):
                    nc.tensor.matmul(
                        psums[h * nq + q],
                        lhsT=a_sb[:, c, h * P:(h + 1) * P],
                        rhs=bp[:, q * 512:(q + 1) * 512],
                        start=first,
                        stop=last,
                    )

        # epilogue: bias add + cast to bf16 + store
        for h in range(mh):
            for q in range(nq):
                ot = out_pool.tile([P, 512], mybir.dt.bfloat16, name="ot")
                nc.vector.tensor_add(
                    out=ot,
                    in0=psums[h * nq + q],
                    in1=bias_sb[:, q * 512:(q + 1) * 512],
                )
                nc.sync.dma_start(
                    out_ap[h * P:(h + 1) * P, q * 512:(q + 1) * 512], ot
                )


def build(kch=KCH, nb=NB, m=M, repeat=1):
    nc = bacc.Bacc(
        "TRN2",
        target_bir_lowering=False,
        debug=False,
        num_devices=NCORES,
    )
    at = nc.dram_tensor("at", [kch * P, m], mybir.dt.bfloat16, kind="ExternalInput").ap()
    bt = nc.dram_tensor("bt", [kch * P, nb], mybir.dt.uint8, kind="ExternalInput").ap()
    sbt = nc.dram_tensor("sbt", [kch * 8, nb], mybir.dt.bfloat16, kind="ExternalInput").ap()
    bias = nc.dram_tensor("bias", [P, nb], mybir.dt.bfloat16, kind="ExternalInput").ap()
    out = nc.dram_tensor("out", [m, nb], mybir.dt.bfloat16, kind="ExternalOutput").ap()
    with tile.TileContext(nc) as tc:
        tile_body(tc, out, at, bt, sbt, bias, kch=kch, nb=nb, m=m, repeat=repeat)
    nc.compile()
    return nc


def marshal(a, a_scale, a_global_scale, b, b_scale, b_global_scale, bias):
    """Host-side input prep. Returns per-core in_maps."""
    a = np.asarray(a)
    a_scale = np.asarray(a_scale, np.float32)
    ga = float(np.asarray(a_global_scale, np.float32))
    b = np.asarray(b)
    b_scale = np.asarray(b_scale, np.float32)
    gb = float(np.asarray(b_global_scale, np.float32))
    bias = np.asarray(bias, np.float32)

    # A side: full dequant (small), fold global scales, transpose to [K, M]
    a_vals = _FP4[_codes(a)]                                   # [M, K]
    a_deq = a_vals.reshape(M, K // BLOCK, BLOCK) * (a_scale * (ga * gb))[..., None]
    at = np.ascontiguousarray(a_deq.reshape(M, K).T).astype(ml_dtypes.bfloat16)

    # B side: decode codes to e4m3 value bytes, transpose to [K, N]
    b_vals_e4m3 = _FP4.astype(ml_dtypes.float8_e4m3)[_codes(b)]   # [N, K] e4m3
    btf = np.ascontiguousarray(b_vals_e4m3.T).view(np.uint8)      # [K, N] u8

    # within-chunk k-row permutation: partition p holds original row
    # (p % 8) * 16 + p // 8, so its scale row is (p % 8)
    perm = k_perm(K // P)
    at = np.ascontiguousarray(at[perm])
    btf = np.ascontiguousarray(btf[perm])
    sbt_f = np.ascontiguousarray(b_scale.T).astype(ml_dtypes.bfloat16)  # [K/16, N]

    in_maps = []
    for ci in range(NCORES):
        sl = slice(ci * NB, (ci + 1) * NB)
        bias_rep = np.ascontiguousarray(
            np.broadcast_to(bias[None, sl], (P, NB))
        ).astype(ml_dtypes.bfloat16)
        in_maps.append({
            "at": at,
            "bt": np.ascontiguousarray(btf[:, sl]),
            "sbt": np.ascontiguousarray(sbt_f[:, sl]),
            "bias": bias_rep,
        })
    return in_maps


_CACHE = {}


def kernel(a, a_scale, a_global_scale, b, b_scale, b_global_scale, bias):
    in_maps = marshal(a, a_scale, a_global_scale, b, b_scale, b_global_scale, bias)
    if "nc" not in _CACHE:
        _CACHE["nc"] = build()
    res = bass_utils.run_bass_kernel_spmd(
        _CACHE["nc"], in_maps, core_ids=list(range(NCORES))
    )
    return np.concatenate([r["out"] for r in res.results], axis=1)
